# revision 33
# baseline (speedup 1.0000x reference)
"""SpGAT_Conv Trainium2 kernel: 8-core SPMD spectral GNN conv.

Math (reference):
    a = softmax(alpha)
    pre = x @ W                                   [N, D]
    out_low  = s0 @ (a0 * (s1 @ pre))             [N, D]
    out_high = s2 @ (a1 * (s3 @ pre))             [N, D]
    out = relu(max(out_low, out_high) + bias)

Sharding (v9, contraction-sharded middle):  let S = concat(s3, s1) row-wise
(high band first).  Core c owns x rows / pre rows [1024c, 1024c+1024) and
out rows [1024c, 1024c+1024):
    phase 1: pre_c = x_c @ W  (32 matmuls; x row-sharded, W replicated)
    phase 2: t_partial = S[:, cols_c] @ pre_c for ALL 8192 t rows -- the
             contraction slice is exactly the locally computed pre_c, so no
             pre gather is ever needed.  64 strip-sequential [128, 512]
             output strips, high band first.
    comm:    3 pipelined (ReduceScatter(+)-then-AllGather) pairs over strip
             groups [0,22) [22,44) [44,64): RS sums the 8 partial t blocks
             and shards them; AG rebroadcasts full t rows.  RS then AG in
             rank order composes to the identity row order.
    phase 3: out_c = relu(max(a0*s0_c@t1 + b, a1*s2_c@t3 + b)): high band
             then low band over t chunks in arrival (o) order, with t chunks
             JIT-loaded from the AG outputs in exact consumption order.

Queue discipline (hard-won): DMA HW queue ring slots form one global
sequence shared by both HWDGE engines, so any DMA blocked on a collective
semaphore stalls every later-slotted DMA.  Hence (a) t chunks are loaded
JIT in consumption order -- whatever such a load stalls needed that same
collective anyway -- and (b) those loads share a tile pool with the phase-2
strip loads so buffer-reuse deps keep them from occupying ring slots until
this core's phase 2 is nearly done.

All big operands are pre-transposed host-side during sharding so the PE's
contraction dim lands on SBUF partitions with plain contiguous DMAs.
Compute dtype is bf16 (host-cast; full PE rate) with fp32 PSUM
accumulation; the ReduceScatter also reduces in bf16.

A dependency-free tiny AllGather is issued at kernel start to absorb the
first-collective init + inter-core launch skew.
"""

import os

import numpy as np

N_CORES = 8
N = 8192
K = 2048
NK = N - K          # 6144
D = 512
ROWS = N // N_CORES  # 1024 rows per core
P = 128
RCH = ROWS // P      # 8  (pre chunks per core / out strips per core)
NCH = N // P         # 64 (t-row strips over full N)
KCH = K // P         # 16 (low-band chunk count; high band = 48)

# phase-2 computes t strips in o-order = high band first:
#   o in [0, 48)  -> global chunk j = o + KCH   (high band, s3 rows)
#   o in [48, 64) -> global chunk j = o - 48    (low band, s1 rows)
def _jmap(o):
    return o + KCH if o < NCH - KCH else o - (NCH - KCH)

# RS+AG strip groups over o (21/21/22 split keeps the last gather, which is
# a hard barrier on the slowest core, as early as possible)
GROUPS = [(0, 22), (22, 44), (44, 64)]

COMPUTE = os.environ.get("SPGAT_COMPUTE", "bf16")  # "bf16" | "f32r"
DEBUG = os.environ.get("SPGAT_DEBUG", "0") == "1"

_CACHE = {}


def _build_nc(compute):
    import concourse.mybir as mybir
    import concourse.tile as tile
    from concourse import bacc

    f32 = mybir.dt.float32
    bf16 = mybir.dt.bfloat16
    f32r = mybir.dt.float32r
    cdt = bf16 if compute == "bf16" else f32   # storage dtype of matmul operands

    def mmcast(ap):
        return ap.bitcast(f32r) if compute == "f32r" else ap

    nc = bacc.Bacc(
        "TRN2", target_bir_lowering=False, debug=False, num_devices=N_CORES
    )

    xt = nc.dram_tensor("xt", [D, ROWS], cdt, kind="ExternalInput").ap()
    w = nc.dram_tensor("w", [D, D], cdt, kind="ExternalInput").ap()
    alpha = nc.dram_tensor("alpha", [2], f32, kind="ExternalInput").ap()
    bias = nc.dram_tensor("bias", [D], f32, kind="ExternalInput").ap()
    # o-major transposed S column-slice: st3[o, cb, p, m] =
    #   S_perm[128 o + m, 1024 c + 128 cb + p]
    st3 = nc.dram_tensor("st3", [NCH, RCH, P, P], cdt, kind="ExternalInput").ap()
    s0t = nc.dram_tensor("s0t", [K, ROWS], cdt, kind="ExternalInput").ap()
    s2t = nc.dram_tensor("s2t", [NK, ROWS], cdt, kind="ExternalInput").ap()
    out = nc.dram_tensor("out", [ROWS, D], f32, kind="ExternalOutput").ap()
    if DEBUG:
        t_dump = nc.dram_tensor("t_dump", [N, D], cdt, kind="ExternalOutput").ap()

    groups = [list(range(N_CORES))]

    with tile.TileContext(nc) as tc:
        with (
            tc.tile_pool(name="const", bufs=1) as const,
            tc.tile_pool(name="small", bufs=1) as small,
            tc.tile_pool(name="strips", bufs=14) as strips,
            tc.tile_pool(name="rstrips", bufs=12) as rstrips,
            tc.tile_pool(name="tstp", bufs=16) as tstp,
            tc.tile_pool(name="stage", bufs=5) as stage,
            tc.tile_pool(name="stash", bufs=1) as stashp,
            tc.tile_pool(name="ps", bufs=8, space="PSUM") as ps,
            tc.tile_pool(name="dram", bufs=1, space="DRAM") as dram,
        ):
            # ---- collective warm-up: absorb first-collective init + launch
            # skew.  No input deps so it triggers immediately.
            warm_in = dram.tile([8, 8], f32, name="warm_in")
            warm_out = dram.tile([64, 8], f32, name="warm_out", addr_space="Shared")
            nc.gpsimd.collective_compute(
                "AllGather",
                mybir.AluOpType.bypass,
                replica_groups=groups,
                ins=[warm_in.opt()],
                outs=[warm_out.opt()],
            )

            # ---- input DMAs: the phase-1 operands first
            w_sb = const.tile([P, D // P, D], cdt, name="w_sb")
            nc.sync.dma_start(w_sb[:], w.rearrange("(c p) d -> p c d", p=P))
            xt_sb = small.tile([P, D // P, ROWS], cdt, name="xt_sb")
            nc.sync.dma_start(xt_sb[:], xt.rearrange("(c p) r -> p c r", p=P))
            asb = const.tile([1, 2], f32, name="asb")
            nc.sync.dma_start(asb[:], alpha[None, :])
            bsb = const.tile([1, D], f32, name="bsb")
            nc.sync.dma_start(bsb[:], bias[None, :])

            # ---- phase 1: pre_c = x_c @ W  (local 1024 rows only)
            pre_sb = small.tile([P, RCH, D], cdt, name="pre_sb")
            for j in range(RCH):
                acc = ps.tile([P, D], f32, name=f"acc1_{j}", tag="acc")
                for dc in range(D // P):
                    nc.tensor.matmul(
                        acc[:],
                        mmcast(xt_sb[:, dc, P * j : P * (j + 1)]),
                        mmcast(w_sb[:, dc, :]),
                        start=(dc == 0),
                        stop=(dc == D // P - 1),
                    )
                if j % 2 == 0:
                    nc.vector.tensor_copy(pre_sb[:, j, :], acc[:])
                else:
                    nc.scalar.copy(pre_sb[:, j, :], acc[:])

            # ---- setup (needed only in phase 3): softmax(alpha); broadcast
            # a, bias/a0, bias/a1 to 128 partitions via ones-matmul.
            amax = const.tile([1, 1], f32, name="amax")
            nc.vector.tensor_tensor(
                amax[:], asb[:, 0:1], asb[:, 1:2], mybir.AluOpType.max
            )
            ash = const.tile([1, 2], f32, name="ash")
            nc.vector.tensor_scalar(
                ash[:], asb[:], amax[:, 0:1], None, mybir.AluOpType.subtract
            )
            aexp = const.tile([1, 2], f32, name="aexp")
            nc.scalar.activation(aexp[:], ash[:], mybir.ActivationFunctionType.Exp)
            asum = const.tile([1, 1], f32, name="asum")
            nc.vector.tensor_tensor(
                asum[:], aexp[:, 0:1], aexp[:, 1:2], mybir.AluOpType.add
            )
            arec = const.tile([1, 1], f32, name="arec")
            nc.vector.reciprocal(arec[:], asum[:])
            afin = const.tile([1, 2], f32, name="afin")
            nc.vector.tensor_scalar(
                afin[:], aexp[:], arec[:, 0:1], None, mybir.AluOpType.mult
            )
            ainv = const.tile([1, 2], f32, name="ainv")   # [1/a0, 1/a1]
            nc.vector.reciprocal(ainv[:], afin[:])
            bd0 = const.tile([1, D], f32, name="bd0")     # bias / a0
            nc.vector.tensor_scalar(
                bd0[:], bsb[:], ainv[:, 0:1], None, mybir.AluOpType.mult
            )
            bd1 = const.tile([1, D], f32, name="bd1")     # bias / a1
            nc.vector.tensor_scalar(
                bd1[:], bsb[:], ainv[:, 1:2], None, mybir.AluOpType.mult
            )

            ones = const.tile([1, P], f32, name="ones")
            nc.vector.memset(ones[:], 1.0)
            ps_a = ps.tile([P, 2], f32, name="ps_a", tag="acc")
            nc.tensor.matmul(ps_a[:], ones[:], afin[:], start=True, stop=True)
            a128 = const.tile([P, 2], f32, name="a128")
            nc.vector.tensor_copy(a128[:], ps_a[:])
            ps_b = ps.tile([P, D], f32, name="ps_b", tag="acc")
            nc.tensor.matmul(ps_b[:], ones[:], bd0[:], start=True, stop=True)
            bd0_128 = const.tile([P, D], f32, name="bd0_128")
            nc.vector.tensor_copy(bd0_128[:], ps_b[:])
            ps_c = ps.tile([P, D], f32, name="ps_c", tag="acc")
            nc.tensor.matmul(ps_c[:], ones[:], bd1[:], start=True, stop=True)
            bd1_128 = const.tile([P, D], f32, name="bd1_128")
            nc.vector.tensor_copy(bd1_128[:], ps_c[:])

            # ---- phase 2: t_partial = S[:, cols_c] @ pre_c, 64 strips in
            # o-order; staging stores go on the scalar HWDGE queue so the
            # sync queue streams the strip loads uninterrupted.
            t_part = dram.tile([NCH * P, D], cdt, name="t_part")
            ag_outs = [
                dram.tile([(o1 - o0) * P, D], cdt, name=f"ar_out{g}")
                for g, (o0, o1) in enumerate(GROUPS)
            ]

            def t_comm(g):
                o0, o1 = GROUPS[g]
                nc.gpsimd.collective_compute(
                    "AllReduce",
                    mybir.AluOpType.add,
                    replica_groups=groups,
                    ins=[t_part[P * o0 : P * o1, :].opt()],
                    outs=[ag_outs[g].opt()],
                )

            trig = {o1 - 1: g for g, (o0, o1) in enumerate(GROUPS)}
            for o in range(NCH):
                sl = strips.tile([P, RCH, P], cdt, name=f"s3_{o}", tag="strip")
                nc.sync.dma_start(sl[:], st3[o].rearrange("cb p m -> p cb m"))
                acc = ps.tile([P, D], f32, name=f"acc2_{o}", tag="acc")
                for cb in range(RCH):
                    nc.tensor.matmul(
                        acc[:],
                        mmcast(sl[:, cb, :]),
                        mmcast(pre_sb[:, cb, :]),
                        start=(cb == 0),
                        stop=(cb == RCH - 1),
                    )
                # staging copy always on Vector: the RS collectives cause
                # ~16us DMA brownouts, and a ring-blocked store on the scalar
                # queue must not stall the copies that drain PSUM (the copy
                # is the strip matmuls' WAR dependency).  The 16-deep tst
                # pool lets the scalar-side stores lag through a brownout.
                tst = tstp.tile([P, D], cdt, name=f"t_st_{o}", tag="st")
                nc.vector.tensor_copy(tst[:], acc[:])
                nc.scalar.dma_start(t_part[P * o : P * (o + 1), :], tst[:])
                if o in trig:
                    t_comm(trig[o])

            # ---- phase 3: out_c = relu(max(a0*s0_c@t1 + b, a1*s2_c@t3 + b))
            # over t chunks in o (arrival) order: high band exactly o<48.
            def t_load(o):
                g = next(g for g, (o0, o1) in enumerate(GROUPS) if o < o1)
                o0 = GROUPS[g][0]
                tq = strips.tile([P, D], cdt, name=f"tq_{o}", tag="strip")
                nc.sync.dma_start(
                    tq[:], ag_outs[g][P * (o - o0) : P * (o - o0 + 1), :]
                )
                if DEBUG:
                    j = _jmap(o)
                    nc.sync.dma_start(t_dump[P * j : P * (j + 1), :], tq[:])
                return tq

            NHI = NCH - KCH  # 48
            accs3 = [
                ps.tile([P, D], f32, name=f"acc3_{nt}", tag="acc")
                for nt in range(RCH)
            ]
            stash = [
                stashp.tile([P, D], f32, name=f"hst_{nt}", tag=f"hst{nt}")
                for nt in range(RCH)
            ]
            for nt in range(RCH):  # PSUM preload: bias/a1 for the high band
                # on Vector only: the scalar queue may still be draining
                # ring-blocked t_part stores at phase-3 start
                nc.vector.tensor_copy(accs3[nt][:], bd1_128[:])
            for o in range(NHI):
                tq = t_load(o)
                jj = _jmap(o) - KCH
                strip = rstrips.tile([P, ROWS], cdt, name=f"rh_{o}", tag="strip")
                nc.sync.dma_start(strip[:], s2t[P * jj : P * (jj + 1), :])
                for nt in range(RCH):
                    nc.tensor.matmul(
                        accs3[nt][:],
                        mmcast(strip[:, P * nt : P * (nt + 1)]),
                        mmcast(tq[:]),
                        start=False,
                        stop=(o == NHI - 1),
                    )
            for nt in range(RCH):
                # stash = a1*acc (+bias via preload), then preload bias/a0
                # for the low band; alternate engines.
                if nt % 2 == 0:
                    nc.vector.tensor_scalar(
                        stash[nt][:], accs3[nt][:], a128[:, 1:2], None,
                        mybir.AluOpType.mult,
                    )
                    nc.vector.tensor_copy(accs3[nt][:], bd0_128[:])
                else:
                    nc.scalar.mul(stash[nt][:], accs3[nt][:], a128[:, 1:2])
                    nc.scalar.copy(accs3[nt][:], bd0_128[:])
            for o in range(NHI, NCH):
                tq = t_load(o)
                j = _jmap(o)
                strip = rstrips.tile([P, ROWS], cdt, name=f"rl_{o}", tag="strip")
                nc.sync.dma_start(strip[:], s0t[P * j : P * (j + 1), :])
                for nt in range(RCH):
                    nc.tensor.matmul(
                        accs3[nt][:],
                        mmcast(strip[:, P * nt : P * (nt + 1)]),
                        mmcast(tq[:]),
                        start=False,
                        stop=(o == NCH - 1),
                    )
            for nt in range(RCH):
                lo = stage.tile([P, D], f32, name=f"elo_{nt}", tag="elo")
                # fused (acc * a0) max stash in one DVE pass
                nc.vector.scalar_tensor_tensor(
                    lo[:], accs3[nt][:], a128[:, 0:1], stash[nt][:],
                    mybir.AluOpType.mult, mybir.AluOpType.max,
                )
                osb = stage.tile([P, D], f32, name=f"osb_{nt}", tag="osb")
                nc.scalar.activation(
                    osb[:], lo[:], mybir.ActivationFunctionType.Relu
                )
                row0 = P * nt
                # alternate HWDGE engines so stores drain on two queues
                if nt % 2 == 0:
                    nc.sync.dma_start(out[row0 : row0 + P, :], osb[:])
                else:
                    nc.scalar.dma_start(out[row0 : row0 + P, :], osb[:])

    nc.compile()
    return nc


def _get_nc(compute):
    if compute not in _CACHE:
        _CACHE[compute] = _build_nc(compute)
    return _CACHE[compute]


def _shard_inputs(x, weights, alpha, bias, s0, s1, s2, s3, compute):
    import ml_dtypes

    cnp = ml_dtypes.bfloat16 if compute == "bf16" else np.float32

    def prep(a):  # transpose + cast, C-contiguous
        return np.ascontiguousarray(a.T).astype(cnp, copy=False)

    alpha = np.ascontiguousarray(alpha, dtype=np.float32)
    bias = np.ascontiguousarray(bias, dtype=np.float32)
    w_p = np.ascontiguousarray(weights).astype(cnp, copy=False)
    in_maps = []
    for c in range(N_CORES):
        r0, r1 = ROWS * c, ROWS * (c + 1)
        # S_perm = [s3; s1] rows (high band first); core c takes the
        # contraction-column slice [r0, r1): st3[o, cb, p, m] =
        # S_perm[128 o + m, r0 + 128 cb + p]
        s_cols = np.concatenate(
            [np.asarray(s3[:, r0:r1]), np.asarray(s1[:, r0:r1])], axis=0
        )  # [8192, 1024]
        st3 = np.ascontiguousarray(
            s_cols.reshape(NCH, P, RCH, P).transpose(0, 2, 3, 1)
        ).astype(cnp, copy=False)
        in_maps.append(
            {
                "xt": prep(x[r0:r1]),
                "w": w_p,
                "alpha": alpha,
                "bias": bias,
                "st3": st3,
                "s0t": prep(s0[r0:r1]),
                "s2t": prep(s2[r0:r1]),
            }
        )
    return in_maps


def kernel(x, weights, alpha, bias, s0, s1, s2, s3, _trace=False):
    from concourse.bass_utils import run_bass_kernel_spmd

    compute = COMPUTE
    nc = _get_nc(compute)
    in_maps = _shard_inputs(
        np.asarray(x), np.asarray(weights), np.asarray(alpha), np.asarray(bias),
        np.asarray(s0), np.asarray(s1), np.asarray(s2), np.asarray(s3), compute,
    )
    kwargs = {}
    if _trace:
        # warm-up execution: compile + collective init + allocator warm so the
        # traced run measures steady-state
        run_bass_kernel_spmd(nc, in_maps, core_ids=list(range(N_CORES)))
        kwargs = dict(trace=True, trace_cores=list(range(N_CORES)))
    r = run_bass_kernel_spmd(nc, in_maps, core_ids=list(range(N_CORES)), **kwargs)
    full = np.concatenate([res["out"] for res in r.results], axis=0)
    if _trace:
        return full, r
    return full


# revision 34
# speedup vs baseline: 1.0428x; 1.0428x over previous
"""SpGAT_Conv Trainium2 kernel: 8-core SPMD spectral GNN conv.

Math (reference):
    a = softmax(alpha)
    pre = x @ W                                   [N, D]
    out_low  = s0 @ (a0 * (s1 @ pre))             [N, D]
    out_high = s2 @ (a1 * (s3 @ pre))             [N, D]
    out = relu(max(out_low, out_high) + bias)

Sharding: row-shard the node dim N across 8 cores.  Let S = concat(s1, s3)
(rows 0..N-1).  Core c owns rows [1024c, 1024c+1024):
    phase 1: pre = x @ W computed fully on every core (replicated; a
             contraction-sharded variant with ReduceScatter was measured
             SLOWER -- the CC stream only delivers ~1MB of t per 26us while
             phase 3 consumes 1MB per 8us, so extra collectives lose)
    phase 2: t_c = S_c @ pre, STRIP-SEQUENTIAL: each 128-row output strip
             runs its full 64-chunk contraction before the next strip;
             groups of strips are AllGathered as soon as their last strip
             stages, so all gathers complete during phase 2/3 compute.
    phase 3: out_c = relu(max(a0*s0_c@t1 + b, a1*s2_c@t3 + b)), high band
             then low band.  bias/a is preloaded into PSUM before each
             band's accumulation (matmul continue-mode adds on top); the
             high stash is relu'd up front (max(X, relu(Y)) == relu(max(X,
             Y)) because relu(Y) >= 0), so the per-strip epilogue is a
             single fused (a0*acc) max stash DVE op straight into the
             output store.

Queue discipline (hard-won): DMA HW queue ring slots form one global
sequence shared by both HWDGE engines, so any DMA blocked on a collective
semaphore stalls every later-slotted DMA.  Hence (a) t chunks are loaded
JIT in exact phase-3 consumption order -- whatever such a load stalls
needed that same collective anyway -- and (b) those loads share a tile
pool with the phase-2 strip loads so buffer-reuse deps keep them from
occupying ring slots until this core's phase 2 is nearly done.

All big operands are pre-transposed host-side during sharding so the PE's
contraction dim lands on SBUF partitions with plain contiguous DMAs.
Compute dtype is bf16 (host-cast; full PE rate) with fp32 PSUM
accumulation; set SPGAT_COMPUTE=f32r for the float32r variant.

A dependency-free tiny AllGather is issued at kernel start to absorb the
first-collective init + inter-core launch skew.
"""

import os

import numpy as np

N_CORES = 8
N = 8192
K = 2048
NK = N - K          # 6144
D = 512
ROWS = N // N_CORES  # 1024 rows per core
P = 128
RCH = ROWS // P      # 8  (row chunks per core / output strips)
NCH = N // P         # 64 (contraction chunks over full N)
KCH = K // P         # 16 (low-band chunks; high band = NCH - KCH = 48)
JB = 8               # contraction chunks per phase-2 strip DMA load

# sub-AllGather strip groups.  Each collective costs ~25-30us nearly
# independent of size (setup/sync dominated), so few groups amortize the
# overhead; 3 groups still lets the first two gathers complete while
# phase 2 is computing.
GROUPS = [[0, 1, 2], [3, 4], [5, 6, 7]]
GBASE = [0, 24, 40]  # q-index base of each group (8 ranks * strips)

COMPUTE = os.environ.get("SPGAT_COMPUTE", "bf16")  # "bf16" | "f32r"
DEBUG = os.environ.get("SPGAT_DEBUG", "0") == "1"

_CACHE = {}

# t-chunk catalog: sub-AG g delivers, for every rank c, its strips GROUPS[g]
# = global chunks j = 8c + s.  Phase 3 consumes chunks in arrival (q) order:
# q = GBASE[g] + c * len(GROUPS[g]) + s_idx.
ARRIVAL = [
    (8 * c + s, GBASE[g] + c * len(GROUPS[g]) + si, g, c)
    for g in range(len(GROUPS))
    for c in range(N_CORES)
    for si, s in enumerate(GROUPS[g])
]


def _build_nc(compute):
    import concourse.mybir as mybir
    import concourse.tile as tile
    from concourse import bacc

    f32 = mybir.dt.float32
    bf16 = mybir.dt.bfloat16
    f32r = mybir.dt.float32r
    cdt = bf16 if compute == "bf16" else f32   # storage dtype of matmul operands

    def mmcast(ap):
        return ap.bitcast(f32r) if compute == "f32r" else ap

    nc = bacc.Bacc(
        "TRN2", target_bir_lowering=False, debug=False, num_devices=N_CORES
    )

    xt = nc.dram_tensor("xt", [D, N], cdt, kind="ExternalInput").ap()
    w = nc.dram_tensor("w", [D, D], cdt, kind="ExternalInput").ap()
    alpha = nc.dram_tensor("alpha", [2], f32, kind="ExternalInput").ap()
    bias = nc.dram_tensor("bias", [D], f32, kind="ExternalInput").ap()
    # strip-major S_c^T: st2[k, j, p, m] = S_c[128k + m, 128j + p]
    st2 = nc.dram_tensor("st2", [RCH, NCH, P, P], cdt, kind="ExternalInput").ap()
    s0t = nc.dram_tensor("s0t", [K, ROWS], cdt, kind="ExternalInput").ap()
    s2t = nc.dram_tensor("s2t", [NK, ROWS], cdt, kind="ExternalInput").ap()
    out = nc.dram_tensor("out", [ROWS, D], f32, kind="ExternalOutput").ap()
    if DEBUG:
        pre_dump = nc.dram_tensor("pre_dump", [N, D], cdt, kind="ExternalOutput").ap()
        t_dump = nc.dram_tensor("t_dump", [N, D], cdt, kind="ExternalOutput").ap()

    groups = [list(range(N_CORES))]

    with tile.TileContext(nc) as tc:
        with (
            tc.tile_pool(name="const", bufs=1) as const,
            tc.tile_pool(name="bigA", bufs=1) as bigA,
            tc.tile_pool(name="xtp", bufs=12) as xtp,
            tc.tile_pool(name="strips", bufs=8) as strips,
            tc.tile_pool(name="rstrips", bufs=12) as rstrips,
            tc.tile_pool(name="stage", bufs=5) as stage,
            tc.tile_pool(name="stash", bufs=1) as stashp,
            tc.tile_pool(name="ps", bufs=8, space="PSUM") as ps,
            tc.tile_pool(name="dram", bufs=1, space="DRAM") as dram,
        ):
            # ---- collective warm-up: absorb first-collective init + launch
            # skew.  No input deps so it triggers immediately.
            warm_in = dram.tile([8, 8], f32, name="warm_in")
            warm_out = dram.tile([64, 8], f32, name="warm_out", addr_space="Shared")
            nc.gpsimd.collective_compute(
                "AllGather",
                mybir.AluOpType.bypass,
                replica_groups=groups,
                ins=[warm_in.opt()],
                outs=[warm_out.opt()],
            )

            # ---- phase-1 input DMAs first: w, then rotating xt chunk loads
            w_sb = const.tile([P, D // P, D], cdt, name="w_sb")
            nc.sync.dma_start(w_sb[:], w.rearrange("(c p) d -> p c d", p=P))
            asb = const.tile([1, 2], f32, name="asb")
            nc.sync.dma_start(asb[:], alpha[None, :])
            bsb = const.tile([1, D], f32, name="bsb")
            nc.sync.dma_start(bsb[:], bias[None, :])

            # ---- phase 1: pre = x @ W, computed fully on every core; xt
            # streams through a small rotating pool (each chunk is consumed
            # exactly once)
            xt_v = xt.rearrange("(c p) n -> p c n", p=P)
            pre_sb = bigA.tile([P, NCH, D], cdt, name="pre_sb", tag="bigA")
            for j in range(NCH):
                xc = xtp.tile([P, D // P, P], cdt, name=f"xc_{j}", tag="xc")
                nc.sync.dma_start(xc[:], xt_v[:, :, P * j : P * (j + 1)])
                acc = ps.tile([P, D], f32, name=f"acc1_{j}", tag="acc")
                for dc in range(D // P):
                    nc.tensor.matmul(
                        acc[:],
                        mmcast(xc[:, dc, :]),
                        mmcast(w_sb[:, dc, :]),
                        start=(dc == 0),
                        stop=(dc == D // P - 1),
                    )
                if j % 2 == 0:  # alternate engines: faster PSUM bank release
                    nc.vector.tensor_copy(pre_sb[:, j, :], acc[:])
                else:
                    nc.scalar.copy(pre_sb[:, j, :], acc[:])

            if DEBUG:
                for j in range(NCH):
                    nc.sync.dma_start(
                        pre_dump[P * j : P * (j + 1), :], pre_sb[:, j, :]
                    )

            # ---- setup (after phase-1 matmul issue; results needed only in
            # phase 3): softmax(alpha); broadcast a, bias/a0, bias/a1 to 128
            # partitions via ones-matmul.
            amax = const.tile([1, 1], f32, name="amax")
            nc.vector.tensor_tensor(
                amax[:], asb[:, 0:1], asb[:, 1:2], mybir.AluOpType.max
            )
            ash = const.tile([1, 2], f32, name="ash")
            nc.vector.tensor_scalar(
                ash[:], asb[:], amax[:, 0:1], None, mybir.AluOpType.subtract
            )
            aexp = const.tile([1, 2], f32, name="aexp")
            nc.scalar.activation(aexp[:], ash[:], mybir.ActivationFunctionType.Exp)
            asum = const.tile([1, 1], f32, name="asum")
            nc.vector.tensor_tensor(
                asum[:], aexp[:, 0:1], aexp[:, 1:2], mybir.AluOpType.add
            )
            arec = const.tile([1, 1], f32, name="arec")
            nc.vector.reciprocal(arec[:], asum[:])
            afin = const.tile([1, 2], f32, name="afin")
            nc.vector.tensor_scalar(
                afin[:], aexp[:], arec[:, 0:1], None, mybir.AluOpType.mult
            )
            ainv = const.tile([1, 2], f32, name="ainv")   # [1/a0, 1/a1]
            nc.vector.reciprocal(ainv[:], afin[:])
            bd0 = const.tile([1, D], f32, name="bd0")     # bias / a0
            nc.vector.tensor_scalar(
                bd0[:], bsb[:], ainv[:, 0:1], None, mybir.AluOpType.mult
            )
            bd1 = const.tile([1, D], f32, name="bd1")     # bias / a1
            nc.vector.tensor_scalar(
                bd1[:], bsb[:], ainv[:, 1:2], None, mybir.AluOpType.mult
            )

            ones = const.tile([1, P], f32, name="ones")
            nc.vector.memset(ones[:], 1.0)
            zeros = const.tile([P, D], f32, name="zeros")
            nc.vector.memset(zeros[:], 0.0)
            ps_a = ps.tile([P, 2], f32, name="ps_a", tag="acc")
            nc.tensor.matmul(ps_a[:], ones[:], afin[:], start=True, stop=True)
            a128 = const.tile([P, 2], f32, name="a128")
            nc.vector.tensor_copy(a128[:], ps_a[:])
            ps_b = ps.tile([P, D], f32, name="ps_b", tag="acc")
            nc.tensor.matmul(ps_b[:], ones[:], bd0[:], start=True, stop=True)
            bd0_128 = const.tile([P, D], f32, name="bd0_128")
            nc.vector.tensor_copy(bd0_128[:], ps_b[:])
            ps_c = ps.tile([P, D], f32, name="ps_c", tag="acc")
            nc.tensor.matmul(ps_c[:], ones[:], bd1[:], start=True, stop=True)
            bd1_128 = const.tile([P, D], f32, name="bd1_128")
            nc.vector.tensor_copy(bd1_128[:], ps_c[:])

            # ---- phase 2: t_c = S_c @ pre, strip-sequential; each group's
            # rows are AllGathered as soon as its last strip stages.
            t_in = dram.tile([ROWS, D], cdt, name="t_in")
            t_outs = [
                dram.tile([P * len(gs) * N_CORES, D], cdt, name=f"t_out{g}",
                          addr_space="Shared")
                for g, gs in enumerate(GROUPS)
            ]

            def t_subag(g):
                gs = GROUPS[g]
                nc.gpsimd.collective_compute(
                    "AllGather",
                    mybir.AluOpType.bypass,
                    replica_groups=groups,
                    ins=[t_in[P * gs[0] : P * (gs[-1] + 1), :].opt()],
                    outs=[t_outs[g].opt()],
                )

            st2_v = st2.rearrange("k (B b) p m -> k B p b m", b=JB)
            NB = NCH // JB  # 8 loads per strip
            for kt in range(RCH):
                acc = ps.tile([P, D], f32, name=f"acc2_{kt}", tag="acc")
                for B in range(NB):
                    sl = strips.tile([P, JB, P], cdt, name=f"s{kt}_{B}",
                                     tag="strip")
                    nc.sync.dma_start(sl[:], st2_v[kt, B])
                    for b in range(JB):
                        j = JB * B + b
                        nc.tensor.matmul(
                            acc[:],
                            mmcast(sl[:, b, :]),
                            mmcast(pre_sb[:, j, :]),
                            start=(j == 0),
                            stop=(j == NCH - 1),
                        )
                tst = stage.tile([P, D], cdt, name=f"t_st_{kt}", tag="st")
                if kt % 2 == 0:
                    nc.vector.tensor_copy(tst[:], acc[:])
                else:
                    nc.scalar.copy(tst[:], acc[:])
                nc.sync.dma_start(t_in[P * kt : P * (kt + 1), :], tst[:])
                if kt in (2, 4, 7):
                    t_subag({2: 0, 4: 1, 7: 2}[kt])

            # ---- phase 3: out_c = relu(max(a0*s0_c@t1 + b, a1*s2_c@t3 + b))
            # t chunks are loaded just-in-time on the sync queue in exact
            # consumption order, from a pool shared with the phase-2 strips.
            def t_load(j, q, g):
                tq = strips.tile([P, D], cdt, name=f"tq_{q}", tag="strip")
                r0 = P * (q - GBASE[g])
                nc.sync.dma_start(tq[:], t_outs[g][r0 : r0 + P, :])
                if DEBUG:
                    nc.sync.dma_start(t_dump[P * j : P * (j + 1), :], tq[:])
                return tq

            HI_CHUNKS = [e for e in ARRIVAL if e[0] >= KCH]
            LO_CHUNKS = [e for e in ARRIVAL if e[0] < KCH]
            accs3 = [
                ps.tile([P, D], f32, name=f"acc3_{nt}", tag="acc")
                for nt in range(RCH)
            ]
            stash = [
                stashp.tile([P, D], f32, name=f"hst_{nt}", tag=f"hst{nt}")
                for nt in range(RCH)
            ]
            for nt in range(RCH):  # PSUM preload: bias/a1 for the high band
                if nt % 2 == 0:
                    nc.vector.tensor_copy(accs3[nt][:], bd1_128[:])
                else:
                    nc.scalar.copy(accs3[nt][:], bd1_128[:])
            for idx, (j, q, g, c) in enumerate(HI_CHUNKS):
                tq = t_load(j, q, g)
                jj = j - KCH
                strip = rstrips.tile([P, ROWS], cdt, name=f"rh_{q}", tag="strip")
                nc.sync.dma_start(strip[:], s2t[P * jj : P * (jj + 1), :])
                for nt in range(RCH):
                    nc.tensor.matmul(
                        accs3[nt][:],
                        mmcast(strip[:, P * nt : P * (nt + 1)]),
                        mmcast(tq[:]),
                        start=False,
                        stop=(idx == len(HI_CHUNKS) - 1),
                    )
            for nt in range(RCH):
                # stash = relu(a1*acc + b) in one fused DVE pass ((acc*a1)
                # max 0 with the bias carried by the PSUM preload), then
                # preload bias/a0 for the low band.
                if nt % 2 == 0:
                    nc.vector.scalar_tensor_tensor(
                        stash[nt][:], accs3[nt][:], a128[:, 1:2], zeros[:],
                        mybir.AluOpType.mult, mybir.AluOpType.max,
                    )
                    nc.vector.tensor_copy(accs3[nt][:], bd0_128[:])
                else:
                    nc.scalar.mul(stash[nt][:], accs3[nt][:], a128[:, 1:2])
                    nc.scalar.activation(
                        stash[nt][:], stash[nt][:],
                        mybir.ActivationFunctionType.Relu,
                    )
                    nc.scalar.copy(accs3[nt][:], bd0_128[:])
            for idx, (j, q, g, c) in enumerate(LO_CHUNKS):
                tq = t_load(j, q, g)
                strip = rstrips.tile([P, ROWS], cdt, name=f"rl_{q}", tag="strip")
                nc.sync.dma_start(strip[:], s0t[P * j : P * (j + 1), :])
                for nt in range(RCH):
                    nc.tensor.matmul(
                        accs3[nt][:],
                        mmcast(strip[:, P * nt : P * (nt + 1)]),
                        mmcast(tq[:]),
                        start=False,
                        stop=(idx == len(LO_CHUNKS) - 1),
                    )
            for nt in range(RCH):
                # epilogue: relu(max(a0*lo + b, a1*hi + b)) == (acc*a0) max
                # stash_r -- stash is already relu'd and >= 0
                lo = stage.tile([P, D], f32, name=f"elo_{nt}", tag="elo")
                nc.vector.scalar_tensor_tensor(
                    lo[:], accs3[nt][:], a128[:, 0:1], stash[nt][:],
                    mybir.AluOpType.mult, mybir.AluOpType.max,
                )
                row0 = P * nt
                # alternate HWDGE engines so stores drain on two queues
                if nt % 2 == 0:
                    nc.sync.dma_start(out[row0 : row0 + P, :], lo[:])
                else:
                    nc.scalar.dma_start(out[row0 : row0 + P, :], lo[:])

    nc.compile()
    return nc


def _get_nc(compute):
    if compute not in _CACHE:
        _CACHE[compute] = _build_nc(compute)
    return _CACHE[compute]


def _shard_inputs(x, weights, alpha, bias, s0, s1, s2, s3, compute):
    import ml_dtypes

    cnp = ml_dtypes.bfloat16 if compute == "bf16" else np.float32

    def prep(a):  # transpose + cast, C-contiguous
        return np.ascontiguousarray(a.T).astype(cnp, copy=False)

    alpha = np.ascontiguousarray(alpha, dtype=np.float32)
    bias = np.ascontiguousarray(bias, dtype=np.float32)
    w_p = np.ascontiguousarray(weights).astype(cnp, copy=False)
    xt_full = prep(x)  # [D, N], replicated to every core
    in_maps = []
    for c in range(N_CORES):
        r0, r1 = ROWS * c, ROWS * (c + 1)
        # S = concat(s1, s3) rows; core c owns rows [r0, r1)
        if r1 <= K:
            s_rows = np.asarray(s1[r0:r1])
        elif r0 >= K:
            s_rows = np.asarray(s3[r0 - K : r1 - K])
        else:  # straddles the boundary (not the case for these shapes)
            s_rows = np.concatenate([s1[r0:], s3[: r1 - K]], axis=0)
        # strip-major transpose: st2[k, j, p, m] = s_rows[128k + m, 128j + p]
        st2 = np.ascontiguousarray(
            s_rows.reshape(RCH, P, NCH, P).transpose(0, 2, 3, 1)
        ).astype(cnp, copy=False)
        in_maps.append(
            {
                "xt": xt_full,
                "w": w_p,
                "alpha": alpha,
                "bias": bias,
                "st2": st2,
                "s0t": prep(s0[r0:r1]),
                "s2t": prep(s2[r0:r1]),
            }
        )
    return in_maps


def kernel(x, weights, alpha, bias, s0, s1, s2, s3, _trace=False):
    from concourse.bass_utils import run_bass_kernel_spmd

    compute = COMPUTE
    nc = _get_nc(compute)
    in_maps = _shard_inputs(
        np.asarray(x), np.asarray(weights), np.asarray(alpha), np.asarray(bias),
        np.asarray(s0), np.asarray(s1), np.asarray(s2), np.asarray(s3), compute,
    )
    kwargs = {}
    if _trace:
        # warm-up execution: compile + collective init + allocator warm so the
        # traced run measures steady-state
        run_bass_kernel_spmd(nc, in_maps, core_ids=list(range(N_CORES)))
        kwargs = dict(trace=True, trace_cores=list(range(N_CORES)))
    r = run_bass_kernel_spmd(nc, in_maps, core_ids=list(range(N_CORES)), **kwargs)
    full = np.concatenate([res["out"] for res in r.results], axis=0)
    if _trace:
        return full, r
    return full
